# revision 1
# baseline (speedup 1.0000x reference)
"""Trainium2 Bass kernel for nn_Attention_77025943487081.

Sharding: batch (4) data-parallel x 2-way head tensor-parallel over 8 cores.
Core c handles batch c//2 and heads [8*(c%2), 8*(c%2)+8). Each core produces a
partial c_proj output (contribution of its 512 input channels); the host adds
the two partials per batch plus the c_proj bias.

The per-head Conv2D projections, cross-head mixes, position projections and
biases are algebraically folded (on host) into dense matrices so the device
only runs plain matmuls:
  q[s, (g,d)] = sum_e x[s,e] * Mq[e,(g,d)] + sum_p pos[s,p] * Mq_p[p,(g,d)] + bq[(g,d)]
The 1/sqrt(D) score scale is folded into the Q-side matrices. All matmul
operands are float32r (fp32 bits, full-rate PE streaming); softmax statistics
ride along as an extra ones-column appended to V, so the PV matmul emits the
denominators in PSUM row 64 for free.
"""

import numpy as np
from contextlib import ExitStack

import concourse.bass as bass
import concourse.tile as tile
from concourse import bacc, mybir
from concourse.bass_utils import run_bass_kernel_spmd

F32 = mybir.dt.float32
F32R = mybir.dt.float32r

B, S, E, H, D, P = 4, 2048, 1024, 16, 64, 64
G = 8            # heads per core
NC = 8           # cores
EC = 9           # contraction chunks: 8 x 128 hidden + 1 (pos+bias, padded)
QKD = G * D      # 512 = per-core q (or k) width
ACT_EXP = mybir.ActivationFunctionType.Exp


def build_nc():
    nc = bacc.Bacc("TRN2", target_bir_lowering=False, debug=False, num_devices=NC)
    xT = nc.dram_tensor("xT", [EC, 128, S], F32R, kind="ExternalInput").ap()
    mqk = nc.dram_tensor("Mqk", [EC, 128, 2 * QKD], F32R, kind="ExternalInput").ap()
    mv = nc.dram_tensor("Mv", [EC, 128, QKD], F32R, kind="ExternalInput").ap()
    wc = nc.dram_tensor("Wc", [4, 128, E], F32R, kind="ExternalInput").ap()
    onesd = nc.dram_tensor("ones", [128, 128], F32R, kind="ExternalInput").ap()
    out = nc.dram_tensor("out", [S, E], F32, kind="ExternalOutput").ap()

    NT = S // 128  # 16 sequence tiles

    with nc.allow_low_precision("float32r staging of matmul operands"), \
         tile.TileContext(nc) as tc, ExitStack() as top:
        # pools alive across phases
        vaug_p = top.enter_context(tc.tile_pool(name="vaug", bufs=1))
        const_p = top.enter_context(tc.tile_pool(name="const", bufs=1))
        qk_p = top.enter_context(tc.tile_pool(name="qkt", bufs=1))

        v_aug = vaug_p.tile([128, NT, G, D + 1], F32R)  # [k-part, s-tile, head, d|ones]
        ones_col = const_p.tile([1, 64], F32R)
        # resident qT/kT: chunks 0-3 = qT head-pairs, 4-7 = kT
        qkt = [qk_p.tile([128, S], F32R, name=f"qkt{m}") for m in range(8)]
        nc.sync.dma_start(out=ones_col, in_=onesd[0:1, 0:64])
        nc.sync.dma_start(out=v_aug[:, :, :, D:D + 1], in_=onesd[:, 0:NT * G])

        # ---------------- phase 1: projections ----------------
        # ss-outer streaming of x^T slices; q/k/v psum accumulated over the 9
        # contraction chunks and copied straight into resident SBUF tiles.
        with tc.tile_pool(name="ph1", bufs=1) as ph1, \
             tc.tile_pool(name="xtss", bufs=2) as xtss_p, \
             tc.tile_pool(name="ps_qk", bufs=8, space="PSUM") as ps_qk:
            mqk_sb = ph1.tile([128, EC, 2 * QKD], F32R)
            mv_sb = ph1.tile([128, EC, QKD], F32R)
            for ec in range(EC):
                nc.sync.dma_start(out=mqk_sb[:, ec, :], in_=mqk[ec])
            for ec in range(EC):
                nc.sync.dma_start(out=mv_sb[:, ec, :], in_=mv[ec])

            for ss in range(4):
                xtss = xtss_p.tile([128, EC, 512], F32R)
                for ec in range(EC):
                    nc.sync.dma_start(out=xtss[:, ec, :],
                                      in_=xT[ec][:, ss * 512:(ss + 1) * 512])
                pss = [ps_qk.tile([128, 512], F32, tag="qk", name=f"qkps{m}")
                       for m in range(8)]
                for ec in range(EC):
                    for m in range(8):
                        nc.tensor.matmul(
                            pss[m][:, :],
                            mqk_sb[:, ec, m * 128:(m + 1) * 128],
                            xtss[:, ec, :],
                            start=(ec == 0), stop=(ec == EC - 1),
                        )
                for m in range(8):
                    nc.vector.tensor_copy(qkt[m][:, ss * 512:(ss + 1) * 512],
                                          pss[m][:, :])
                # V in natural [s, (g,d)] layout, into v_aug (col D = ones)
                for sti in range(4):
                    stt = ss * 4 + sti
                    pv = ps_qk.tile([128, 512], F32, tag="qk", name=f"vps{sti}")
                    for ec in range(EC):
                        nc.tensor.matmul(
                            pv[:, :],
                            xtss[:, ec, sti * 128:(sti + 1) * 128],
                            mv_sb[:, ec, :],
                            start=(ec == 0), stop=(ec == EC - 1),
                        )
                    nc.vector.tensor_copy(
                        v_aug[:, stt, :, 0:D],
                        pv[:, :].rearrange("p (g d) -> p g d", g=G),
                    )

        # ---------------- phase 2: attention ----------------
        with tc.tile_pool(name="oT", bufs=1) as oT_p:
            oT = oT_p.tile([128, 4, S], F32R)  # [(2 heads)*64 part, head-pair, q]
            with tc.tile_pool(name="pt", bufs=4) as pt_p, \
                 tc.tile_pool(name="rcp", bufs=4) as rcp_p, \
                 tc.tile_pool(name="ps_st", bufs=4, space="PSUM") as ps_st, \
                 tc.tile_pool(name="ps_o", bufs=1, space="PSUM") as ps_o:
                for h in range(G):
                    m, half = h // 2, h % 2
                    qt = qkt[m][64 * half:64 * half + 64, :]
                    kt = qkt[4 + m][64 * half:64 * half + 64, :]
                    po = ps_o.tile([65, S], F32)
                    for kc in range(NT):
                        q0 = kc * 128
                        ptile = pt_p.tile([128, S], F32R)
                        # scores^T + exp in <=512-wide chunks for deep pipelining
                        c0 = q0
                        while c0 < S:
                            c1 = min(S, (c0 // 512 + 1) * 512)
                            stp = ps_st.tile([128, 512], F32, tag="st")
                            nc.tensor.matmul(
                                stp[:, 0:c1 - c0],
                                kt[:, q0:q0 + 128],
                                qt[:, c0:c1],
                                start=True, stop=True,
                            )
                            nc.scalar.activation(
                                ptile[:, c0 - q0:c1 - q0],
                                stp[:, 0:c1 - c0],
                                ACT_EXP,
                            )
                            c0 = c1
                        # causal mask on the diagonal 128x128 block (separate tile so
                        # only the small diagonal PV matmul waits on it): keep q >= k
                        diag = pt_p.tile([128, 128], F32R, tag="diag")
                        nc.gpsimd.affine_select(
                            out=diag[:, :], in_=ptile[:, 0:128],
                            compare_op=mybir.AluOpType.is_ge,
                            fill=0.0, base=0, pattern=[[1, 128]], channel_multiplier=-1,
                        )
                        # PV accumulation (+ softmax denominator in row 64)
                        nc.tensor.matmul(
                            po[:, q0:q0 + 128], v_aug[:, kc, h, :], diag[:, :],
                            start=(kc == 0), stop=True,
                            skip_group_check=True,
                        )
                        for qb in range(kc // 4, 4):
                            qs = max(qb * 512, q0 + 128)
                            n = (qb + 1) * 512 - qs
                            if n <= 0:
                                continue
                            nc.tensor.matmul(
                                po[:, qs:qs + n],
                                v_aug[:, kc, h, :],
                                ptile[:, qs - q0:qs - q0 + n],
                                start=(kc == 0), stop=(kc == qb * 4 + 3),
                                skip_group_check=True,
                            )
                        # normalize each 512-q group as soon as its last k-chunk
                        # landed: oT[d, q] = po[d, q] / po[64, q]
                        if kc % 4 == 3:
                            qg = kc // 4
                            sl = slice(qg * 512, (qg + 1) * 512)
                            rcp = rcp_p.tile([1, 512], F32R)
                            nc.vector.reciprocal(rcp, po[64:65, sl])
                            bc_ps = ps_st.tile([64, 512], F32, tag="st", name=f"bcps{qg}")
                            nc.tensor.matmul(bc_ps[:, :], ones_col, rcp,
                                             start=True, stop=True)
                            bcst = rcp_p.tile([64, 512], F32R, tag="bcast",
                                              name=f"bcast{qg}")
                            nc.vector.tensor_copy(bcst[:, :], bc_ps[:, :])
                            nc.vector.tensor_mul(
                                oT[64 * half:64 * half + 64, m, sl],
                                po[0:64, sl], bcst[:, :],
                            )

            # ---------------- phase 3: partial c_proj ----------------
            with tc.tile_pool(name="wc", bufs=1) as wc_p, \
                 tc.tile_pool(name="ostage", bufs=3) as ostage_p, \
                 tc.tile_pool(name="ps_c", bufs=3, space="PSUM") as ps_c:
                wc_sb = wc_p.tile([128, 4, E], F32R)
                for gc in range(4):
                    nc.sync.dma_start(out=wc_sb[:, gc, :], in_=wc[gc])
                for stt in range(NT):
                    pc = ps_c.tile([128, E], F32)
                    for gc in range(4):
                        for ee in range(2):
                            nc.tensor.matmul(
                                pc[:, ee * 512:(ee + 1) * 512],
                                oT[:, gc, stt * 128:(stt + 1) * 128],
                                wc_sb[:, gc, ee * 512:(ee + 1) * 512],
                                start=(gc == 0), stop=(gc == 3),
                            )
                    ost = ostage_p.tile([128, E], F32)
                    nc.vector.tensor_copy(ost[:, :], pc[:, :])
                    nc.sync.dma_start(out=out[stt * 128:(stt + 1) * 128, :], in_=ost[:, :])

    nc.compile()
    return nc


def prep_core_inputs(hidden_states, position_states, Wq, bq, Wqh, bqh, Wk, bk,
                     Wkh, bkh, Wv, bv, Wvh, bvh, Wp, bp, Wpe, bpe, Wc, bc):
    """Build the per-core input maps (host-side weight folding + sharding)."""
    f32 = np.float32

    def fused(parity):
        hs = slice(G * parity, G * parity + G)
        mats = {}
        for name, (Wa, ba, Wh, bh, v) in {
            "q": (Wq, bq, Wqh[hs], bqh[hs], 0),
            "k": (Wk, bk, Wkh[hs], bkh[hs], 1),
            "v": (Wv, bv, Wvh[hs], bvh[hs], 2),
        }.items():
            mx = np.einsum("hed,ghd->hegd", Wa, Wh).reshape(E, QKD)
            mp = np.einsum("pd,g->pgd", Wp[:, v * D:(v + 1) * D], Wpe[v, 0, hs]).reshape(P, QKD)
            bias = (np.einsum("hd,ghd->gd", ba, Wh) + bh
                    + bp[v * D:(v + 1) * D][None, :] * Wpe[v, 0, hs][:, None]
                    + bpe[hs][:, None]).reshape(QKD)
            if name == "q":
                sc = 1.0 / np.sqrt(np.float32(D))
                mx, mp, bias = mx * sc, mp * sc, bias * sc
            mats[name] = (mx, mp, bias)
        def chunks(mx, mp, bias):
            w = mx.shape[1]
            m9 = np.zeros((EC, 128, w), f32)
            m9[:8] = mx.reshape(8, 128, w)
            m9[8, :P] = mp
            m9[8, P] = bias
            return m9
        mqk9 = np.concatenate([chunks(*mats["q"]), chunks(*mats["k"])], axis=2)
        mv9 = chunks(*mats["v"])
        wc4 = np.ascontiguousarray(
            Wc.reshape(H, D, E)[hs].reshape(QKD, E).reshape(4, 128, E).astype(f32))
        return np.ascontiguousarray(mqk9), np.ascontiguousarray(mv9), wc4

    per_parity = [fused(0), fused(1)]
    ones = np.ones((128, 128), f32)

    in_maps = []
    for c in range(NC):
        b, parity = c // 2, c % 2
        x9 = np.zeros((EC, 128, S), f32)
        x9[:8] = np.ascontiguousarray(hidden_states[b].T).reshape(8, 128, S)
        x9[8, :P] = position_states[b].T
        x9[8, P] = 1.0
        mqk9, mv9, wc4 = per_parity[parity]
        in_maps.append({"xT": x9, "Mqk": mqk9, "Mv": mv9, "Wc": wc4, "ones": ones})
    return in_maps


_NC_CACHE = {}


def get_nc():
    if "nc" not in _NC_CACHE:
        _NC_CACHE["nc"] = build_nc()
    return _NC_CACHE["nc"]


def kernel(**inputs):
    nc = get_nc()
    in_maps = prep_core_inputs(**inputs)
    res = run_bass_kernel_spmd(nc, in_maps, list(range(NC)))
    bc = inputs["bc"]
    outs = [res.results[2 * b]["out"] + res.results[2 * b + 1]["out"] + bc
            for b in range(B)]
    return np.stack(outs).astype(np.float32)



# revision 5
# speedup vs baseline: 1.1587x; 1.1587x over previous
"""Trainium2 Bass kernel for nn_Attention_77025943487081.

Sharding: batch (4) data-parallel x 2-way head tensor-parallel over 8 cores.
Core c handles batch c//2 and heads [8*(c%2), 8*(c%2)+8). Each core emits 4
partial c_proj outputs (one per head-pair, bf16); the host sums the 8 partials
per batch and adds the c_proj bias.

Numerics (validated in numpy against the f32 reference, rel err ~6e-3 vs the
2e-2 gate):
  - The folded per-head+cross-head projection matrices are I + C with C at
    0.02 scale.  q/k/v = bf16(x_slice) + (x8 + dx8) @ C8 / 32 where x8/dx8
    are fp8e4m3 value+residual and C8 = fp8(32*C) (the 32x pre-scale keeps
    C's entries out of fp8's subnormal range). The correction matmuls run in
    fp8 DoubleRow mode (256-deep contraction, 0.5 cycles/row).
  - Scores/PV/c_proj operands are bf16, accumulation always f32 PSUM.
  - The 1/sqrt(D) score scale is applied inside the softmax exp activation
    (out = exp(in * 0.125)), so q is staged unscaled.

Attention uses a transposed PV: ptile holds exp(scores)^T [k, q] per k-chunk
and PV computes out[q, d] with q as the PSUM partition dim (full 128-row
output vs 65 in the [d, q] orientation); softmax denominators come from
1-column matmuls against a ones vector. Normalization is then a cheap
per-partition DVE multiply, and a PE transpose puts o back into [d, s]
layout for c_proj.

Schedule: the Act engine paces each head (~19us of softmax exp vs ~12us of
PE work), so leftover phase-1 chains and the previous pair's partial c_proj
are emitted as filler inside later heads' k-chunk loops to keep the PE and
DVE busy during Act-paced stretches. The final pair's c_proj evacuates PSUM
on both DVE and Act (Act is idle by then).
"""

import numpy as np
import ml_dtypes
from contextlib import ExitStack

import concourse.bass as bass
import concourse.tile as tile
from concourse import bacc, mybir
from concourse.bass_utils import run_bass_kernel_spmd

F32 = mybir.dt.float32
BF16 = mybir.dt.bfloat16
FP8 = mybir.dt.float8e4
DR = mybir.MatmulPerfMode.DoubleRow
ACT_EXP = mybir.ActivationFunctionType.Exp
ACT_COPY = mybir.ActivationFunctionType.Copy
MULT = mybir.AluOpType.mult
ADD = mybir.AluOpType.add

B, S, E, H, D, P = 4, 2048, 1024, 16, 64, 64
G = 8            # heads per core
NCORE = 8
NT = S // 128    # 16 sequence tiles
NPAIR = 5        # fp8 DoubleRow contraction pairs: 1280 = 5*256 rows (1089 used)
CSCALE = 32.0    # fp8 pre-scale on the correction matrices
INV_CS = 1.0 / CSCALE


def build_nc():
    nc = bacc.Bacc("TRN2", target_bir_lowering=False, debug=False, num_devices=NCORE)
    cqk8 = nc.dram_tensor("cqk8", [NPAIR, 128, 2, 1024], FP8, kind="ExternalInput").ap()
    cv8 = nc.dram_tensor("cv8", [NPAIR, 128, 2, 512], FP8, kind="ExternalInput").ap()
    xt8 = nc.dram_tensor("xt8", [NPAIR, 128, 2, S], FP8, kind="ExternalInput").ap()
    dxt8 = nc.dram_tensor("dxt8", [4, 128, 2, S], FP8, kind="ExternalInput").ap()
    xtid = nc.dram_tensor("xtid", [4, 128, S], BF16, kind="ExternalInput").ap()
    xnat = nc.dram_tensor("xnat", [NT, 128, 512], BF16, kind="ExternalInput").ap()
    wc16 = nc.dram_tensor("wc16", [4, 128, E], BF16, kind="ExternalInput").ap()
    ident = nc.dram_tensor("ident", [128, 128], BF16, kind="ExternalInput").ap()
    outp = nc.dram_tensor("outp", [4, S, E], BF16, kind="ExternalOutput").ap()

    with nc.allow_low_precision("bf16/fp8 staged operands; f32 PSUM accumulation"), \
         tile.TileContext(nc) as tc, ExitStack() as top:
        const_p = top.enter_context(tc.tile_pool(name="const", bufs=1))
        qk_p = top.enter_context(tc.tile_pool(name="qkt", bufs=1))
        vaug_p = top.enter_context(tc.tile_pool(name="vaug", bufs=1))
        oT_p = top.enter_context(tc.tile_pool(name="oT", bufs=1))

        cqk_sb = const_p.tile([128, NPAIR, 2, 1024], FP8)
        cv_sb = const_p.tile([128, NPAIR, 2, 512], FP8)
        xt_sb = const_p.tile([128, NPAIR, 2, S], FP8)
        dxt_sb = const_p.tile([128, 4, 2, S], FP8)
        xtid_sb = const_p.tile([128, 4, S], BF16)
        xnat_sb = const_p.tile([128, NT, 512], BF16)
        wc_sb = const_p.tile([128, 4, E], BF16)
        id_sb = const_p.tile([128, 128], BF16)
        ones_col = const_p.tile([128, 1], BF16)

        qkt = [qk_p.tile([128, S], BF16, name=f"qkt{m}") for m in range(8)]
        v_sb = vaug_p.tile([128, NT, G, 64], BF16)
        oT = oT_p.tile([128, 4, S], BF16)

        # startup-ordered DMAs: the first qk chain only needs pair 0 of
        # cqk8/xt8, so interleave by pair
        for pr in range(NPAIR):
            nc.sync.dma_start(out=cqk_sb[:, pr], in_=cqk8[pr])
            nc.sync.dma_start(out=xt_sb[:, pr], in_=xt8[pr])
            if pr < 4:
                nc.sync.dma_start(out=dxt_sb[:, pr], in_=dxt8[pr])
        nc.sync.dma_start(out=xtid_sb[:, 0], in_=xtid[0])
        for pr in range(NPAIR):
            nc.sync.dma_start(out=cv_sb[:, pr], in_=cv8[pr])
        for m4 in range(1, 4):
            nc.sync.dma_start(out=xtid_sb[:, m4], in_=xtid[m4])
        for stt in range(NT):
            nc.sync.dma_start(out=xnat_sb[:, stt], in_=xnat[stt])
        nc.sync.dma_start(out=id_sb, in_=ident)
        for gc in range(4):
            nc.sync.dma_start(out=wc_sb[:, gc], in_=wc16[gc])
        nc.vector.memset(ones_col, 1.0)

        with tc.tile_pool(name="stp", bufs=2, space="PSUM") as stp_p, \
             tc.tile_pool(name="pop", bufs=1, space="PSUM") as po_p, \
             tc.tile_pool(name="denp", bufs=1, space="PSUM") as den_p, \
             tc.tile_pool(name="tpp", bufs=1, space="PSUM") as tp_p, \
             tc.tile_pool(name="pt", bufs=2) as pt_p, \
             tc.tile_pool(name="diag", bufs=2) as diag_p, \
             tc.tile_pool(name="on", bufs=2) as on_p, \
             tc.tile_pool(name="rcp", bufs=2) as rcp_p, \
             tc.tile_pool(name="ost", bufs=3) as ost_p:

            # ---------- phase-1 building blocks ----------
            def qk_chain(m, ss):
                """qkt[m][:, ss*512:+512] = identity x-slice + fp8 correction."""
                ps = stp_p.tile([128, 1024], F32, tag="stp", name=f"qkps{m}_{ss}")
                side = 0 if m < 4 else 512
                col0 = side + (m % 4) * 128
                xsl = slice(ss * 512, (ss + 1) * 512)
                for pr in range(NPAIR):
                    nc.tensor.matmul(ps[:, 0:512],
                                     cqk_sb[:, pr, :, col0:col0 + 128],
                                     xt_sb[:, pr, :, xsl],
                                     start=(pr == 0), stop=False, perf_mode=DR)
                for pr in range(4):
                    nc.tensor.matmul(ps[:, 0:512],
                                     cqk_sb[:, pr, :, col0:col0 + 128],
                                     dxt_sb[:, pr, :, xsl],
                                     start=False, stop=(pr == 3), perf_mode=DR)
                nc.vector.scalar_tensor_tensor(
                    out=qkt[m][:, xsl], in0=ps[:, 0:512], scalar=INV_CS,
                    in1=xtid_sb[:, m % 4, xsl], op0=MULT, op1=ADD)

            def v_chain(stt):
                pv = stp_p.tile([128, 1024], F32, tag="stp", name=f"vps{stt}")
                for pr in range(NPAIR):
                    nc.tensor.matmul(pv[:, 0:512],
                                     xt_sb[:, pr, :, stt * 128:(stt + 1) * 128],
                                     cv_sb[:, pr, :, :],
                                     start=(pr == 0), stop=(pr == NPAIR - 1),
                                     perf_mode=DR)
                nc.vector.scalar_tensor_tensor(
                    out=v_sb[:, stt, :, :],
                    in0=pv[:, 0:512].rearrange("p (g d) -> p g d", g=G),
                    scalar=INV_CS,
                    in1=xnat_sb[:, stt].rearrange("p (g d) -> p g d", g=G),
                    op0=MULT, op1=ADD)

            def cproj_chunk(gc, stt, on_act=False):
                """one 128-row slab of the partial c_proj for head-pair gc"""
                pc = stp_p.tile([128, 1024], F32, tag="stp", name=f"pc{gc}_{stt}")
                for ee in range(2):
                    nc.tensor.matmul(pc[:, ee * 512:(ee + 1) * 512],
                                     oT[:, gc, stt * 128:(stt + 1) * 128],
                                     wc_sb[:, gc, ee * 512:(ee + 1) * 512],
                                     start=True, stop=True)
                ost = ost_p.tile([128, E], BF16, tag="ost", name=f"ost{gc}_{stt}")
                if on_act:
                    nc.scalar.activation(ost[:, :], pc[:, :], ACT_COPY)
                else:
                    nc.vector.tensor_copy(ost[:, :], pc[:, :])
                nc.sync.dma_start(out=outp[gc, stt * 128:(stt + 1) * 128, :],
                                  in_=ost[:, :])

            # ---------- phase-2 per-head attention ----------
            def head_attention(h, filler):
                """filler: thunks emitted between k-chunks so the PE/DVE have
                work while the Act engine paces the softmax."""
                m, half = h // 2, h % 2
                qt = qkt[m][64 * half:64 * half + 64, :]
                kt = qkt[4 + m][64 * half:64 * half + 64, :]
                po = po_p.tile([128, 1024], F32, tag="po", name=f"po{h}")
                den = den_p.tile([128, 512], F32, tag="den", name=f"den{h}")
                nc.vector.memset(po[:, :], 0.0)
                nc.vector.memset(den[:, 0:16], 0.0)
                for kc in range(NT):
                    q0 = 128 * kc
                    ptile = pt_p.tile([128, 2048], BF16, tag="pt", name=f"pt{h}_{kc}")
                    for c0 in range(q0, S, 1024):
                        cw = min(1024, S - c0)
                        st = stp_p.tile([128, 1024], F32, tag="stp", name=f"st{h}_{kc}_{c0}")
                        for u0 in range(c0, c0 + cw, 512):
                            uw = min(512, c0 + cw - u0)
                            nc.tensor.matmul(st[:, u0 - c0:u0 - c0 + uw],
                                             kt[:, q0:q0 + 128], qt[:, u0:u0 + uw],
                                             start=True, stop=True)
                        nc.scalar.activation(ptile[:, c0 - q0:c0 - q0 + cw],
                                             st[:, 0:cw], ACT_EXP, scale=0.125)
                    dg = diag_p.tile([128, 128], BF16, tag="dg", name=f"dg{h}_{kc}")
                    nc.gpsimd.affine_select(
                        out=dg, in_=ptile[:, 0:128],
                        compare_op=mybir.AluOpType.is_ge,
                        fill=0.0, base=0, pattern=[[1, 128]], channel_multiplier=-1)
                    for qc in range(kc, NT):
                        lhs = dg[:, :] if qc == kc else \
                            ptile[:, (qc - kc) * 128:(qc - kc + 1) * 128]
                        nc.tensor.matmul(po[:, qc * 64:(qc + 1) * 64], lhs,
                                         v_sb[:, kc, h, :],
                                         start=False, stop=(qc == kc),
                                         skip_group_check=True)
                        nc.tensor.matmul(den[:, qc:qc + 1], lhs, ones_col[:, :],
                                         start=False, stop=(qc == kc),
                                         skip_group_check=True)
                    if filler and kc in (1, 4, 7, 10):
                        filler.pop(0)()
                # normalize: o_n[q, qc, d] = po[q, qc*64+d] / den[q, qc]
                rcp = rcp_p.tile([128, 16], F32, tag="rcp", name=f"rcp{h}")
                o_n = on_p.tile([128, 16, 64], BF16, tag="on", name=f"on{h}")
                nc.vector.reciprocal(rcp, den[:, 0:16])
                nc.vector.tensor_mul(
                    o_n[:, :, :],
                    po[:, :].rearrange("p (q d) -> p q d", q=16),
                    rcp[:, :, None].broadcast_to([128, 16, 64]))
                # transpose to [d, s] for c_proj; odd heads land on partitions
                # 64:128 (matmul tile_position inferred from base partition)
                for jg in range(4):
                    tp = tp_p.tile([128, 4, 128], BF16, tag="tp", name=f"tp{h}_{jg}")
                    for j in range(4):
                        nc.tensor.transpose(tp[64 * half:64 * half + 64, j, :],
                                            o_n[:, jg * 4 + j, :], id_sb[:, :])
                    nc.vector.tensor_copy(
                        oT[64 * half:64 * half + 64, m,
                           jg * 512:(jg + 1) * 512].rearrange(
                               "p (a b) -> p a b", a=4),
                        tp[64 * half:64 * half + 64, :, :])

            # ---------- schedule ----------
            # minimal prefix so head 0's softmax starts early
            for ss in range(4):
                qk_chain(0, ss)
                qk_chain(4, ss)
            for stt in range(4):
                v_chain(stt)

            def v_thunk(s0):
                return lambda: [v_chain(stt) for stt in range(s0, s0 + 3)]

            def qk_thunk(mlo, ss):
                return lambda: [qk_chain(mlo, ss), qk_chain(mlo + 4, ss)]

            def cp_thunk(gc, t0):
                return lambda: [cproj_chunk(gc, stt) for stt in range(t0, t0 + 4)]

            fillers = {
                0: [v_thunk(s0) for s0 in (4, 7, 10, 13)],
                1: [qk_thunk(1, ss) for ss in range(4)],
                2: [cp_thunk(0, 0), cp_thunk(0, 4), qk_thunk(2, 0), qk_thunk(2, 1)],
                3: [cp_thunk(0, 8), cp_thunk(0, 12), qk_thunk(2, 2), qk_thunk(2, 3)],
                4: [cp_thunk(1, 0), cp_thunk(1, 4), qk_thunk(3, 0), qk_thunk(3, 1)],
                5: [cp_thunk(1, 8), cp_thunk(1, 12), qk_thunk(3, 2), qk_thunk(3, 3)],
                6: [cp_thunk(2, 0), cp_thunk(2, 4)],
                7: [cp_thunk(2, 8), cp_thunk(2, 12)],
            }
            for h in range(8):
                head_attention(h, fillers[h])
            # tail: last pair's c_proj with PSUM evacuation split DVE/Act
            for stt in range(NT):
                cproj_chunk(3, stt, on_act=(stt % 2 == 1))

    nc.compile()
    return nc


def _fp8(a, scale=1.0):
    return (np.asarray(a, np.float32) * scale).astype(ml_dtypes.float8_e4m3)


def _bf16(a):
    return np.asarray(a, np.float32).astype(ml_dtypes.bfloat16)


def prep_core_inputs(hidden_states, position_states, Wq, bq, Wqh, bqh, Wk, bk,
                     Wkh, bkh, Wv, bv, Wvh, bvh, Wp, bp, Wpe, bpe, Wc, bc):
    """Host-side weight folding + per-core staging."""
    f32 = np.float32
    eyeE = np.eye(E, dtype=f32)

    def fold(parity):
        hs = slice(G * parity, G * parity + G)
        csl = slice(512 * parity, 512 * parity + 512)
        mats = {}
        for name, (Wa, ba, Wh, bh, v) in {
            "q": (Wq, bq, Wqh[hs], bqh[hs], 0),
            "k": (Wk, bk, Wkh[hs], bkh[hs], 1),
            "v": (Wv, bv, Wvh[hs], bvh[hs], 2),
        }.items():
            mx = np.einsum("hed,ghd->hegd", Wa, Wh).reshape(E, 512)
            mp = np.einsum("pd,g->pgd", Wp[:, v * D:(v + 1) * D],
                           Wpe[v, 0, hs]).reshape(P, 512)
            bias = (np.einsum("hd,ghd->gd", ba, Wh) + bh
                    + bp[v * D:(v + 1) * D][None, :] * Wpe[v, 0, hs][:, None]
                    + bpe[hs][:, None]).reshape(512)
            C = np.zeros((NPAIR * 256, 512), f32)
            C[:E] = mx - eyeE[:, csl]
            C[E:E + P] = mp
            C[E + P] = bias
            mats[name] = C
        cqk = np.concatenate([mats["q"], mats["k"]], axis=1)     # [1280, 1024]
        cqk8 = _fp8(cqk.reshape(NPAIR, 2, 128, 1024).transpose(0, 2, 1, 3), CSCALE)
        cv8 = _fp8(mats["v"].reshape(NPAIR, 2, 128, 512).transpose(0, 2, 1, 3), CSCALE)
        wc = Wc.reshape(H, D, E)[hs].reshape(512, E).reshape(4, 128, E)
        return (np.ascontiguousarray(cqk8), np.ascontiguousarray(cv8),
                np.ascontiguousarray(_bf16(wc)))

    per_parity = [fold(0), fold(1)]
    ident = _bf16(np.eye(128, dtype=f32))

    in_maps = []
    for c in range(NCORE):
        b, parity = c // 2, c % 2
        csl = slice(512 * parity, 512 * parity + 512)
        xaug = np.zeros((NPAIR * 256, S), f32)
        xaug[:E] = hidden_states[b].T
        xaug[E:E + P] = position_states[b].T
        xaug[E + P] = 1.0
        x8 = _fp8(xaug)
        dx = xaug[:E] - x8[:E].astype(f32)
        xt8 = np.ascontiguousarray(
            x8.reshape(NPAIR, 2, 128, S).transpose(0, 2, 1, 3))
        dxt8 = np.ascontiguousarray(
            _fp8(dx).reshape(4, 2, 128, S).transpose(0, 2, 1, 3))
        xtid = np.ascontiguousarray(
            _bf16(hidden_states[b].T[csl]).reshape(4, 128, S))
        xnat = np.ascontiguousarray(
            _bf16(hidden_states[b][:, csl]).reshape(NT, 128, 512))
        cqk8, cv8, wc = per_parity[parity]
        in_maps.append({"cqk8": cqk8, "cv8": cv8, "xt8": xt8, "dxt8": dxt8,
                        "xtid": xtid, "xnat": xnat, "wc16": wc, "ident": ident})
    return in_maps


_NC_CACHE = {}


def get_nc():
    if "nc" not in _NC_CACHE:
        _NC_CACHE["nc"] = build_nc()
    return _NC_CACHE["nc"]


def assemble(results, bc):
    outs = []
    for b in range(B):
        acc = np.zeros((S, E), np.float32)
        for p in range(2):
            acc += results[2 * b + p]["outp"].astype(np.float32).sum(axis=0)
        outs.append(acc + bc)
    return np.stack(outs).astype(np.float32)


def kernel(**inputs):
    nc = get_nc()
    in_maps = prep_core_inputs(**inputs)
    res = run_bass_kernel_spmd(nc, in_maps, list(range(NCORE)))
    return assemble(res.results, inputs["bc"])
